# revision 38
# baseline (speedup 1.0000x reference)
"""Trainium2 Bass kernel for the bipartite GNN recommender (8 NeuronCores).

Redesigned layout (v2):
- Node j -> core j%8. Per-core user rows l=j//8 in [0,25088), products
  l=25088+(p//8) in [25088,37760). Graph edges only touch nodes <200000
  (the reference never offsets prod_idx), so products are self-loop-only
  and their whole chain (proj->conv1->conv2->ts) is computed locally in
  phase P1 with zero collective traffic.
- Conv tables are block-laid per (half, core): user table row for node j:
  l<12544 -> c*12544+l, else 100352+c*12544+(l-12544). One AllGather per
  half, triggered as soon as that half's tiles are produced (overlaps
  compute). Products in ts table at 200704+c*12672+(p//8).
- Scatter segment-sum via transposed one-hot matmuls: stationary = the
  64-col message tile (half the LDWEIGHTS cost), stream = the one-hot,
  accumulate [64,128] per tile in one [64,512] PSUM bank per group.
- Self-term from a persistent SBUF copy of the core's own table slice
  (no indirect gather), dis applied via a persistent [64,25088] bf16
  broadcast table, biases via per-partition activation bias columns.
- Final pair-MLP: |W2| folded into the ts tables (pos/neg dim split on
  host), so per edge: gather t,s -> add -> relu -> two strided reduces
  -> subtract -> sigmoid(scale)*5.
"""
import ml_dtypes
import numpy as np

from concourse import bass, mybir, tile
from concourse.bass import AP, IndirectOffsetOnAxis
from concourse.bass_utils import run_bass_kernel_spmd
from concourse.masks import make_identity
from concourse.tile import add_dep_helper

F32 = mybir.dt.float32
BF16 = mybir.dt.bfloat16
F8 = mybir.dt.float8e4
I32 = mybir.dt.int32

AF = mybir.ActivationFunctionType
ALU = mybir.AluOpType

N_CORES = 8
NU, NP, NE = 200000, 100000, 1000000
SHARD = 37760
PU = 25088            # user rows per core
PC = 12672            # product rows per core
P0 = 12544            # rows per user half per core
UH2 = 8 * P0          # 100352
PROD_BASE = 2 * UH2   # 200704
TAB = PROD_BASE + 8 * PC  # 302080
TILES_C = 99
W32 = 32              # scatter dest-tile width (rows per one-hot tile)
NT = PU // W32        # 784 dest tiles per core
NG = PU // 512        # 49 groups of 16 tiles
EPT = NE // N_CORES
NCH = 984             # pred output cols (125000 edges -> 977, pad to mult of 8)
# user-table bands: AllGathers are split 4 ways and pipelined behind the
# producers; band edges align to 512-row groups (12/12/12/13 groups).
B_OFF = np.array([0, 6144, 12288, 18432, 25088])
B_TAB = 8 * B_OFF     # band base rows in the gathered tables
GEND = [12, 24, 36, 49]  # producing group count per band prefix


# --------------------------------------------------------------------------
# legalization: this walrus build allows at most 1 sync wait per instruction
# --------------------------------------------------------------------------
def _split_sync_waits(nc, max_waits=1):
    import bass_rust
    for bb in nc.main_func.blocks:
        out = []
        for inst in bb.instructions:
            si = inst.sync_info
            if si is not None and si.on_wait is not None and len(si.on_wait) > max_waits:
                waits = list(si.on_wait)
                keep, extra = waits[-max_waits:], waits[:-max_waits]
                while extra:
                    chunk, extra = extra[:max_waits], extra[max_waits:]
                    nop = bass_rust.InstNoOp(name=f"I-{nc.next_id()}", ins=[], outs=[])
                    nop.engine = inst.engine
                    nop.bass_nofuse = True
                    nop.sync_info = mybir.SyncInfo(on_wait=chunk, on_update=[])
                    nc.register_instruction(nop, overwrite=True)
                    out.append(nop)
                si.on_wait = keep
            out.append(inst)
        del bb.instructions[:]
        for i in out:
            bb.add_instruction(i)


# --------------------------------------------------------------------------
# host-side sharding / layout prep
# --------------------------------------------------------------------------
def _pi_user(j):
    j = np.asarray(j, np.int64)
    c, l = j % 8, j // 8
    b = np.searchsorted(B_OFF, l, side="right") - 1
    size = B_OFF[b + 1] - B_OFF[b]
    return (B_TAB[b] + c * size + (l - B_OFF[b])).astype(np.int32)


def _pi_prod(p):
    p = np.asarray(p, np.int64)
    return (PROD_BASE + (p % 8) * PC + p // 8).astype(np.int32)


def _prepare(inputs):
    ei = np.asarray(inputs["edge_index"])
    u_idx = ei[0].astype(np.int64)
    p_idx = ei[1].astype(np.int64)

    # directed messages: src -> dst; both endpoints are node ids < 200000
    src = np.concatenate([u_idx, p_idx])
    dst = np.concatenate([p_idx, u_idx])
    core = (dst % 8).astype(np.int64)
    l = (dst // 8).astype(np.int64)
    src_pi = _pi_user(src)

    order = np.argsort(core * (1 << 32) + l, kind="stable")
    core_s, l_s, srcpi_s = core[order], l[order], src_pi[order]
    core_starts = np.searchsorted(core_s, np.arange(N_CORES + 1))

    # per-tile chunk counts, shared across cores (SPMD: one program)
    cnt = np.bincount(core_s * NT + (l_s // W32),
                      minlength=N_CORES * NT).reshape(N_CORES, NT)
    chunks_t = np.maximum(1, np.ceil(cnt.max(0) / 128).astype(np.int64))
    cb = np.zeros(NT + 1, np.int64)
    np.cumsum(chunks_t, out=cb[1:])
    TC = int(cb[-1])
    # per-group chunk ranges (16 tiles of 32 rows per 512-row group)
    gb = cb[::16]                      # [NG+1] group chunk base
    MAXCH = int(np.max(gb[1:] - gb[:-1]))

    fw = np.asarray(inputs["user_features"], np.float32)
    pw = np.asarray(inputs["product_features"], np.float32)
    ue = np.asarray(inputs["user_emb"], np.float32)
    pe = np.asarray(inputs["product_emb"], np.float32)
    b_uf = np.asarray(inputs["b_uf"], np.float32)
    b_pf = np.asarray(inputs["b_pf"], np.float32)

    pi_u = _pi_user(u_idx)
    pi_p = _pi_prod(p_idx)

    # pred-MLP folding: permute hidden dims so W2>=0 dims come first,
    # scale W1 columns (and pb1) by |W2|*G, recover with sigmoid scale 1/G.
    W1 = np.asarray(inputs["pred_W1"], np.float32)     # [128, 64]
    w2 = np.asarray(inputs["pred_W2"], np.float32).reshape(64)
    pb1 = np.asarray(inputs["pred_b1"], np.float32)
    perm = np.argsort(w2 < 0, kind="stable")           # positives first
    npos = int((w2 >= 0).sum())
    aw = np.abs(w2[perm])
    amax = max(aw.max(), 1e-30)
    G = 1.0 / amax
    colscale = aw * G                                  # in (0, 1]
    W1s = W1[:, perm] * colscale[None, :]
    pb1s = pb1[perm] * colscale
    # negate the w2<0 columns: the stored value v' = -v, so the edge
    # contribution -relu(v) = min(v', 0) and the +/- reduction collapses
    # into ONE contiguous sum (relu on [:npos], min0 on [npos:]).
    W1s[:, npos:] *= -1.0
    pb1s[npos:] *= -1.0
    inv_g = float(amax)                                # sigmoid scale

    per_core = []
    for c in range(N_CORES):
        s0, s1 = core_starts[c], core_starts[c + 1]
        lc, sc = l_s[s0:s1], srcpi_s[s0:s1]
        # flat per-tile chunk layout: tile t owns chunks [cb[t], cb[t+1])
        t = lc // W32
        start = np.searchsorted(t, np.arange(NT))
        pos = np.arange(len(t)) - start[t]
        assert pos.max() < (cb[t + 1] - cb[t]).max() * 128 + 128
        ch = cb[t] + (pos >> 7)
        rows = np.zeros((128, TC), np.int32)
        colv = np.full((128, TC), -1, np.int64)
        rows[pos & 127, ch] = sc
        colv[pos & 127, ch] = lc & (W32 - 1)
        S4 = (colv[:, :, None] == np.arange(W32)
              ).astype(ml_dtypes.float8_e4m3).reshape(128, TC * W32)

        featT = np.zeros((128, SHARD), np.float32)
        embT = np.zeros((64, SHARD), np.float32)
        featT[:, :25000] = fw[c::8].T
        featT[:, 25088:37588] = pw[c::8].T
        embT[:, :25000] = ue[c::8].T + b_uf[:, None]
        embT[:, 25088:37588] = pe[c::8].T + b_pf[:, None]
        embT[:, 25000:25088] = b_uf[:, None]
        embT[:, 37588:] = b_pf[:, None]

        deg = np.bincount(lc, minlength=PU).astype(np.float32)
        dis = 1.0 / np.sqrt(deg + 1.0)
        disTu = np.tile(dis[None, :], (64, 1)).astype(ml_dtypes.bfloat16)

        e0 = c * EPT
        # sort this core's pair-edges by the band of their U-endpoint; a P7
        # block whose edges only touch early bands can start as soon as those
        # band AllGathers complete.
        pu_c = pi_u[e0:e0 + EPT]
        pp_c = pi_p[e0:e0 + EPT]
        uband = np.searchsorted(B_TAB, pu_c, side="right") - 1
        eorder = np.argsort(uband, kind="stable")
        pu_c, pp_c = pu_c[eorder], pp_c[eorder]
        uband_s = uband[eorder]
        blk_band_c = [int(uband_s[min((b + 1) * 4096, EPT) - 1])
                      for b in range((NCH // 8 + 3) // 4)]
        offU = np.zeros((128, NCH), np.int32)
        offP = np.zeros((128, NCH), np.int32)
        el = np.arange(EPT)
        offU[el % 128, el // 128] = pu_c
        offP[el % 128, el // 128] = pp_c
        offUP = np.zeros((128, 2 * NCH), np.int32)
        for g in range(NCH // 8):
            offUP[:, 16 * g:16 * g + 8] = offU[:, 8 * g:8 * g + 8]
            offUP[:, 16 * g + 8:16 * g + 16] = offP[:, 8 * g:8 * g + 8]

        per_core.append(dict(
            featT=featT.astype(ml_dtypes.float8_e4m3), embT=embT.astype(ml_dtypes.bfloat16),
            disTu=disTu, rows=rows, S4=S4,
            offUP=offUP, _colv=colv, _eorder=eorder,
            _blkband=blk_band_c,
        ))

    shared = dict(
        Wuf=np.asarray(inputs["W_uf"], np.float32).astype(ml_dtypes.bfloat16),
        Wpf=np.asarray(inputs["W_pf"], np.float32).astype(ml_dtypes.bfloat16),
        W1c=np.asarray(inputs["conv1_W"], np.float32).astype(ml_dtypes.bfloat16),
        W2c=np.asarray(inputs["conv2_W"], np.float32).astype(ml_dtypes.bfloat16),
        pW1t=np.ascontiguousarray(W1s[:64]).astype(ml_dtypes.bfloat16),
        pW1b=np.ascontiguousarray(W1s[64:]).astype(ml_dtypes.bfloat16),
        b1col=np.asarray(inputs["conv1_b"], np.float32).reshape(64, 1),
        b2col=np.asarray(inputs["conv2_b"], np.float32).reshape(64, 1),
        pb1col=pb1s.reshape(64, 1).astype(np.float32),
        b2pred=np.full((128, 1), float(np.asarray(inputs["pred_b2"]).reshape(())), np.float32),
    )
    blkband = [max(pc["_blkband"][b] for pc in per_core)
               for b in range(len(per_core[0]["_blkband"]))]
    meta = dict(chunks_t=chunks_t.tolist(), cb=cb.tolist(), gb=gb.tolist(),
                TC=TC, MAXCH=MAXCH, npos=npos, inv_g=inv_g, blkband=blkband)
    return per_core, shared, meta


# --------------------------------------------------------------------------
# numpy simulator of the device program (for host-side validation only)
# --------------------------------------------------------------------------
def _simulate(inputs):
    f8 = lambda x: np.asarray(x, np.float32).astype(ml_dtypes.float8_e4m3).astype(np.float32)
    bf = lambda x: np.asarray(x, np.float32).astype(ml_dtypes.bfloat16).astype(np.float32)
    per_core, shared, meta = _prepare(inputs)
    npos, inv_g = meta["npos"], meta["inv_g"]
    cb = np.asarray(meta["cb"])
    Wuf, Wpf = bf(shared["Wuf"]), bf(shared["Wpf"])
    W1c, W2c = bf(shared["W1c"]), bf(shared["W2c"])
    pW1t, pW1b = bf(shared["pW1t"]), bf(shared["pW1b"])
    b1, b2 = shared["b1col"][:, 0], shared["b2col"][:, 0]
    pb1 = shared["pb1col"][:, 0]

    def band_write(tbl, c, arr):
        for b in range(4):
            sz = B_OFF[b + 1] - B_OFF[b]
            tbl[B_TAB[b] + c * sz: B_TAB[b] + (c + 1) * sz] = \
                arr[B_OFF[b]:B_OFF[b + 1]]

    y1_t = np.zeros((PROD_BASE, 64), np.float32)
    ts_t = np.zeros((TAB, 64), np.float32)
    y1ownT, disT, featsT, embsT = [], [], [], []
    for c in range(N_CORES):
        pc = per_core[c]
        ft, et = f8(pc["featT"]), bf(pc["embT"])
        dis = bf(pc["disTu"])[0]  # [PU]
        x0 = bf(ft.T @ Wuf + et.T)              # [SHARD, 64] (user cols valid)
        y1 = f8((x0[:PU] @ W1c) * dis[:, None])
        y1ownT.append(y1)
        disT.append(dis)
        featsT.append(ft)
        embsT.append(et)
        band_write(y1_t, c, y1)
        # region C local chain
        x0c = bf(ft[:, PU:].T @ Wpf + et[:, PU:].T)
        x1c = np.maximum(bf(x0c @ W1c) + b1, 0.0)
        x2c = bf(bf(x1c) @ W2c) + b2
        tsc = f8(bf(x2c) @ pW1b)
        ts_t[PROD_BASE + c * PC: PROD_BASE + (c + 1) * PC] = tsc

    def conv(y_t, layer):
        y2_t = np.zeros((PROD_BASE, 64), np.float32)
        outs = []
        for c in range(N_CORES):
            pc = per_core[c]
            dis = disT[c]
            aggT = np.zeros((64, PU), np.float32)
            rows, colv = pc["rows"], pc["_colv"]
            msg = f8(y_t[rows])                  # [128, TC, 64]
            for t in range(NT):
                acc = np.zeros((64, W32), np.float32)
                for ch in range(cb[t], cb[t + 1]):
                    S = (colv[:, ch:ch + 1] == np.arange(W32)[None, :]
                         ).astype(np.float32)
                    acc += msg[:, ch].T @ S
                aggT[:, t * W32:(t + 1) * W32] = acc
            own = y1ownT[c] if layer == 1 else yown2[c]
            agg = aggT.T + own
            x = bf(agg * dis[:, None])
            if layer == 1:
                x1 = bf(np.maximum(x + b1, 0.0))
                y2 = f8(bf(x1 @ W2c) * dis[:, None])
                outs.append(y2)
                band_write(y2_t, c, y2)
            else:
                x2 = bf(x + b2)
                ts = f8(bf(x2 @ pW1t) + pb1)
                outs.append(ts)
                band_write(ts_t, c, ts)
        return y2_t, outs

    yown2 = None
    y2_t, yown2 = conv(y1_t, 1)
    _, _ = conv(y2_t, 2)

    # P7
    out = np.zeros(NE, np.float32)
    ei = np.asarray(inputs["edge_index"])
    pi_u = _pi_user(ei[0].astype(np.int64))
    pi_p = _pi_prod(ei[1].astype(np.int64))
    t = ts_t[pi_u]
    s = ts_t[pi_p]  # per-edge (device order differs, result order identical)
    v = bf(t + s)
    h = np.concatenate([np.maximum(v[:, :npos], 0.0),
                        np.minimum(v[:, npos:], 0.0)], axis=1)
    logit = h.sum(1)
    z = logit * inv_g + float(np.asarray(inputs["pred_b2"]).reshape(()))
    out[:] = 5.0 / (1.0 + np.exp(-z))
    return out


# --------------------------------------------------------------------------
# device program
# --------------------------------------------------------------------------
def _v3(ap, mid, inner, mid_stride=None, inner_stride=0):
    a = ap.ap
    ms = a[1][0] if mid_stride is None else mid_stride
    return AP(ap.tensor, ap.offset, [list(a[0]), [ms, mid], [inner_stride, inner]])


def _o3(ap, nsub):
    return AP(ap.tensor, ap.offset, [list(ap.ap[0]), [128, nsub], [1, 128]])


def build_program(meta):
    chunks_t = meta["chunks_t"]
    cb = meta["cb"]
    gb = meta["gb"]
    TC, MAXCH = meta["TC"], meta["MAXCH"]
    npos, inv_g = meta["npos"], meta["inv_g"]
    blkband = meta["blkband"]
    nc = bass.Bass("TRN2", target_bir_lowering=False, debug=False, num_devices=N_CORES)

    dp = nc.declare_dram_parameter
    featT_d = dp("featT", [128, SHARD], F8, isOutput=False)
    embT_d = dp("embT", [64, SHARD], BF16, isOutput=False)
    disTu_d = dp("disTu", [64, PU], BF16, isOutput=False)
    rows_d = dp("rows", [128, TC], I32, isOutput=False)
    S4_d = dp("S4", [128, TC * W32], F8, isOutput=False)
    offUP_d = dp("offUP", [128, 2 * NCH], I32, isOutput=False)
    Wuf_d = dp("Wuf", [128, 64], BF16, isOutput=False)
    Wpf_d = dp("Wpf", [128, 64], BF16, isOutput=False)
    W1c_d = dp("W1c", [64, 64], BF16, isOutput=False)
    W2c_d = dp("W2c", [64, 64], BF16, isOutput=False)
    pW1t_d = dp("pW1t", [64, 64], BF16, isOutput=False)
    pW1b_d = dp("pW1b", [64, 64], BF16, isOutput=False)
    b1col_d = dp("b1col", [64, 1], F32, isOutput=False)
    b2col_d = dp("b2col", [64, 1], F32, isOutput=False)
    pb1col_d = dp("pb1col", [64, 1], F32, isOutput=False)
    b2pred_d = dp("b2pred", [128, 1], F32, isOutput=False)
    preds_d = dp("preds", [128, NCH], F32, isOutput=True)

    with tile.TileContext(nc) as tc:
        with tc.tile_pool(name="const", bufs=1) as cp, \
             tc.tile_pool(name="sb", bufs=3) as sb, \
             tc.tile_pool(name="ps", bufs=2, space="PSUM") as ps, \
             tc.tile_pool(name="pssc", bufs=2, space="PSUM") as pssc, \
             tc.tile_pool(name="pst", bufs=2, space="PSUM") as pst:

            def reg_dge(h):
                mloc = nc.lookup_mloc(h)
                if mloc.table_entry_id is None:
                    mloc.table_entry_id = len(nc.dge_table) + 1
                    nc.dge_table.append(mloc.name)
                return h

            ag1_in = reg_dge(nc.dram_tensor("ag1_in", [PU, 64], F8))
            ag2_in = reg_dge(nc.dram_tensor("ag2_in", [PU, 64], F8))
            ag3_in = reg_dge(nc.dram_tensor("ag3_in", [SHARD, 64], F8))
            y1_t = reg_dge(nc.dram_tensor("y1_t", [PROD_BASE, 64], F8, addr_space="Shared"))
            y2_t = reg_dge(nc.dram_tensor("y2_t", [PROD_BASE, 64], F8, addr_space="Shared"))
            ts_t = reg_dge(nc.dram_tensor("ts_t", [TAB, 64], F8, addr_space="Shared"))

            # ---- constants ----
            idn = cp.tile([128, 128], F32, tag="idn")
            make_identity(nc, idn[:])
            idn8 = cp.tile([128, 128], F8, tag="idn8")
            nc.vector.tensor_copy(out=idn8[:], in_=idn[:])
            idn_b = cp.tile([128, 128], BF16, tag="idn_b")
            nc.vector.tensor_copy(out=idn_b[:], in_=idn[:])

            Wuf = cp.tile([128, 64], BF16, tag="Wuf")
            nc.sync.dma_start(out=Wuf[:], in_=Wuf_d[:])
            Wpf = cp.tile([128, 64], BF16, tag="Wpf")
            nc.sync.dma_start(out=Wpf[:], in_=Wpf_d[:])
            W1c = cp.tile([64, 64], BF16, tag="W1c")
            nc.sync.dma_start(out=W1c[:], in_=W1c_d[:])
            W2c = cp.tile([64, 64], BF16, tag="W2c")
            nc.sync.dma_start(out=W2c[:], in_=W2c_d[:])
            pW1t = cp.tile([64, 64], BF16, tag="pW1t")
            nc.sync.dma_start(out=pW1t[:], in_=pW1t_d[:])
            pW1b = cp.tile([64, 64], BF16, tag="pW1b")
            nc.sync.dma_start(out=pW1b[:], in_=pW1b_d[:])
            b1col = cp.tile([64, 1], F32, tag="b1col")
            nc.sync.dma_start(out=b1col[:], in_=b1col_d[:])
            b2col = cp.tile([64, 1], F32, tag="b2col")
            nc.sync.dma_start(out=b2col[:], in_=b2col_d[:])
            pb1col = cp.tile([64, 1], F32, tag="pb1col")
            nc.sync.dma_start(out=pb1col[:], in_=pb1col_d[:])
            b2pred = cp.tile([128, 1], F32, tag="b2pred")
            nc.sync.dma_start(out=b2pred[:], in_=b2pred_d[:])
            # disTu loaded in band pieces interleaved with the first P1 groups
            # so the big transfer does not delay the P1 pipeline start.
            disTu = cp.tile([64, PU], BF16, tag="disTu")
            nc.sync.dma_start(out=disTu[:, :B_OFF[1]], in_=disTu_d[:, :B_OFF[1]])
            rows_all = cp.tile([128, TC], I32, tag="rows_all")
            yown = cp.tile([128, PU], F8, tag="yown")  # rows 0:64 = y1, 64:128 = y2

            def transpose_scatter(srcT, g, dram_out, row0, ident=None):
                """srcT [64, g*128] -> row-major rows [row0, row0+g*128)."""
                trp = pst.tile([128, 256], F32, tag="trp")
                for q in range(g):
                    # transpose via a regular matmul: out = srcT_slice^T @ I
                    nc.tensor.matmul(out=trp[:, q * 64:(q + 1) * 64],
                                     lhsT=srcT[:, q * 128:(q + 1) * 128],
                                     rhs=(ident if ident is not None else idn8)[:64, :64],
                                     start=True, stop=True)
                nnm = sb.tile([128, 256], F8, tag="nnm", bufs=3)
                nc.scalar.activation(out=nnm[:, :g * 64], in_=trp[:, :g * 64], func=AF.Copy)
                d = nc.sync.dma_start(
                    out=AP(dram_out[:].tensor, row0 * 64,
                           [[64, 128], [8192, g], [1, 64]]),
                    in_=AP(nnm[:].tensor, nnm[:].offset,
                           [list(nnm[:].ap[0]), [64, g], [1, 64]]),
                )
                return d

            # ======= P1: user projection + y1 table, P1C interleaved ==========
            sc3c = []

            def emit_p1c_group(g):
                gt = min(4, TILES_C - g * 4)
                col0 = PU + g * 4 * 128
                w = gt * 128
                ft = sb.tile([128, 512], F8, tag="p1c_ft", bufs=3)
                nc.sync.dma_start(out=ft[:, :w], in_=featT_d[:, col0:col0 + w])
                et = sb.tile([64, 512], BF16, tag="p1c_et", bufs=3)
                nc.sync.dma_start(out=et[:, :w], in_=embT_d[:, col0:col0 + w])
                x0p = ps.tile([64, 512], F32, tag="psA")
                nc.tensor.matmul(out=x0p[:, :w], lhsT=Wpf[:], rhs=ft[:, :w], start=True, stop=True)
                x0s = sb.tile([64, 512], BF16, tag="p1c_x0s", bufs=3)
                nc.vector.tensor_add(out=x0s[:, :w], in0=x0p[:, :w], in1=et[:, :w])
                x1p = ps.tile([64, 512], F32, tag="psB", bufs=1)
                nc.tensor.matmul(out=x1p[:, :w], lhsT=W1c[:], rhs=x0s[:, :w], start=True, stop=True)
                x1r = sb.tile([64, 512], BF16, tag="p1c_x1r", bufs=3)
                nc.scalar.activation(out=x1r[:, :w], in_=x1p[:, :w], func=AF.Relu, bias=b1col[:])
                x2p = ps.tile([64, 512], F32, tag="psA")
                nc.tensor.matmul(out=x2p[:, :w], lhsT=W2c[:], rhs=x1r[:, :w], start=True, stop=True)
                x2s = sb.tile([64, 512], BF16, tag="p1c_x2s", bufs=3)
                nc.scalar.activation(out=x2s[:, :w], in_=x2p[:, :w], func=AF.Identity, bias=b2col[:])
                tsp = ps.tile([64, 512], F32, tag="psB", bufs=1)
                nc.tensor.matmul(out=tsp[:, :w], lhsT=pW1b[:], rhs=x2s[:, :w], start=True, stop=True)
                tsc = sb.tile([64, 512], F8, tag="p1c_tsc", bufs=3)
                nc.scalar.activation(out=tsc[:, :w], in_=tsp[:, :w], func=AF.Copy)
                d = transpose_scatter(tsc[:, :w], gt, ag3_in, col0)
                sc3c.append(d)

            sc1 = []  # scatter per group
            for g in range(49):
                col0 = g * 512
                w = 512
                ft = sb.tile([128, 512], F8, tag="p1_ft", bufs=3)
                nc.sync.dma_start(out=ft[:], in_=featT_d[:, col0:col0 + w])
                et = sb.tile([64, 512], BF16, tag="p1_et", bufs=3)
                nc.sync.dma_start(out=et[:], in_=embT_d[:, col0:col0 + w])
                if 1 <= g <= 3:  # stream the rest of disTu behind the pipeline
                    nc.sync.dma_start(out=disTu[:, B_OFF[g]:B_OFF[g + 1]],
                                      in_=disTu_d[:, B_OFF[g]:B_OFF[g + 1]])
                x0p = ps.tile([64, 512], F32, tag="psA")
                nc.tensor.matmul(out=x0p[:], lhsT=Wuf[:], rhs=ft[:], start=True, stop=True)
                x0s = sb.tile([64, 512], BF16, tag="p1_x0s", bufs=3)
                nc.vector.tensor_add(out=x0s[:], in0=x0p[:], in1=et[:])
                y1p = ps.tile([64, 512], F32, tag="psB", bufs=1)
                nc.tensor.matmul(out=y1p[:], lhsT=W1c[:], rhs=x0s[:], start=True, stop=True)
                y1s = sb.tile([64, 512], BF16, tag="p1_y1s", bufs=3)
                nc.vector.tensor_tensor(out=y1s[:], in0=y1p[:],
                                        in1=disTu[:, col0:col0 + w], op=ALU.mult)
                nc.scalar.activation(out=yown[0:64, col0:col0 + w], in_=y1s[:],
                                     func=AF.Copy)
                d = transpose_scatter(y1s[:], 4, ag1_in, col0, ident=idn_b)
                sc1.append(d)

            # conv scatter offsets + P7 offsets: loaded during the AG1 window
            drows = nc.sync.dma_start(out=rows_all[:], in_=rows_d[:])

            def allgather(src, r0, r1, dst, o0, scatters):
                cc = nc.gpsimd.collective_compute(
                    "AllGather", ALU.bypass,
                    ins=[src[r0:r1, :]],
                    outs=[dst[o0:o0 + N_CORES * (r1 - r0), :]],
                    replica_groups=[list(range(N_CORES))],
                )
                for s in scatters:
                    add_dep_helper(cc.ins, s.ins, sync=True, reason="AG reads scatters")
                return cc

            # band AllGathers fire as soon as their producer groups finish
            cc1 = [allgather(ag1_in, B_OFF[b], B_OFF[b + 1], y1_t, B_TAB[b],
                             sc1[:GEND[b]]) for b in range(4)]
            # product chain groups 0-14 fill the AG1 wait window; the rest
            # interleave into conv1 (engines there are not saturated)
            for g in range(15):
                emit_p1c_group(g)

            # ================= conv passes =================
            # per-tile chunk counts (32-row dest tiles, 16 tiles per group)
            def conv_pass(yt, yprevT, layer, ag_out, cc_dep, hooks=None):
                scatters = []
                hist = {}
                for g in range(NG):
                    c0 = gb[g]
                    ncols = gb[g + 1] - c0
                    msg = sb.tile([128, MAXCH * 64], F8, tag="cv_msg", bufs=6)
                    gm = nc.gpsimd.indirect_dma_start(
                        out=msg[:, :ncols * 64], out_offset=None,
                        in_=yt[:],
                        in_offset=IndirectOffsetOnAxis(
                            ap=rows_all[:, c0:c0 + ncols], axis=0),
                    )
                    add_dep_helper(gm.ins, drows.ins, sync=True,
                                   reason="gather reads offsets")
                    for _c in cc_dep:
                        add_dep_helper(gm.ins, _c.ins, sync=True,
                                       reason="gather after AG")
                    if (g - 6) in hist:
                        add_dep_helper(gm.ins, hist[g - 6].ins, sync=True,
                                       reason="WAR msg slot reuse")
                    # host-built one-hot matrices, streamed from DRAM
                    s4a = sb.tile([128, MAXCH * W32], F8, tag="cv_s4", bufs=5)
                    nc.sync.dma_start(out=s4a[:, :ncols * W32],
                                      in_=S4_d[:, c0 * W32:(c0 + ncols) * W32])
                    scp = pssc.tile([64, 512], F32, tag="cv_scp", bufs=3)
                    mm = None
                    mmi = 0
                    msgt, msgo = msg[:].tensor, msg[:].offset
                    s4t, s4o = s4a[:].tensor, s4a[:].offset
                    mpart = list(msg[:].ap[0])
                    spart = list(s4a[:].ap[0])
                    for t in range(16):
                        nchv = chunks_t[16 * g + t]
                        tb = cb[16 * g + t] - c0
                        for j2 in range(nchv // 2):
                            ch = tb + 2 * j2
                            mm = nc.tensor.matmul(
                                out=scp[:, t * W32:(t + 1) * W32],
                                lhsT=AP(msgt, msgo + ch * 64,
                                        [mpart, [64, 2], [1, 64]]),
                                rhs=AP(s4t, s4o + ch * W32,
                                       [spart, [W32, 2], [1, W32]]),
                                start=(j2 == 0),
                                stop=(j2 == nchv // 2 - 1 and nchv % 2 == 0),
                                perf_mode=mybir.MatmulPerfMode.DoubleRow,
                            )
                            if mmi < 2:
                                # the wait gates LDWEIGHTS prefetch too; once
                                # two in-order matmuls carry it, the rest of
                                # the group is implicitly ordered after the
                                # gather (engine queues are in-order).
                                add_dep_helper(mm.ins, gm.ins, sync=True,
                                               reason="matmul reads gathered msg")
                            mmi += 1
                        if nchv % 2:
                            ch = tb + nchv - 1
                            mm = nc.tensor.matmul(
                                out=scp[:, t * W32:(t + 1) * W32],
                                lhsT=AP(msgt, msgo + ch * 64, [mpart, [1, 64]]),
                                rhs=AP(s4t, s4o + ch * W32, [spart, [1, W32]]),
                                start=(nchv == 1), stop=True,
                            )
                            if mmi < 2:
                                add_dep_helper(mm.ins, gm.ins, sync=True,
                                               reason="matmul reads gathered msg")
                            mmi += 1
                    hist[g] = mm
                    col0 = g * 512
                    w = 512
                    t1 = sb.tile([64, 512], F32, tag="cv_t1", bufs=3)
                    nc.vector.tensor_add(out=t1[:, :w], in0=scp[:, :w],
                                         in1=yprevT(col0, w))
                    t2 = sb.tile([64, 512], BF16, tag="cv_t2", bufs=3)
                    nc.vector.tensor_tensor(out=t2[:, :w], in0=t1[:, :w],
                                            in1=disTu[:, col0:col0 + w], op=ALU.mult)
                    if layer == 1:
                        x1r = sb.tile([64, 512], BF16, tag="cv_x1r", bufs=3)
                        nc.scalar.activation(out=x1r[:, :w], in_=t2[:, :w],
                                             func=AF.Relu, bias=b1col[:])
                        y2p = ps.tile([64, 512], F32, tag="psB", bufs=1)
                        nc.tensor.matmul(out=y2p[:, :w], lhsT=W2c[:], rhs=x1r[:, :w],
                                         start=True, stop=True)
                        y2s = sb.tile([64, 512], BF16, tag="cv_y2s", bufs=3)
                        nc.vector.tensor_tensor(out=y2s[:, :w], in0=y2p[:, :w],
                                                in1=disTu[:, col0:col0 + w], op=ALU.mult)
                        nc.scalar.activation(out=yown[64:128, col0:col0 + w],
                                             in_=y2s[:, :w], func=AF.Copy)
                        d = transpose_scatter(y2s[:, :w], 4, ag_out, col0, ident=idn_b)
                    else:
                        x2s = sb.tile([64, 512], BF16, tag="cv_x2s", bufs=3)
                        nc.scalar.activation(out=x2s[:, :w], in_=t2[:, :w],
                                             func=AF.Identity, bias=b2col[:])
                        tsp = ps.tile([64, 512], F32, tag="psB", bufs=1)
                        nc.tensor.matmul(out=tsp[:, :w], lhsT=pW1t[:], rhs=x2s[:, :w],
                                         start=True, stop=True)
                        tsu = sb.tile([64, 512], F8, tag="cv_tsu", bufs=3)
                        nc.scalar.activation(out=tsu[:, :w], in_=tsp[:, :w],
                                             func=AF.Identity, bias=pb1col[:])
                        d = transpose_scatter(tsu[:, :w], 4, ag_out, col0)
                    scatters.append(d)
                    if hooks:
                        for h in hooks.get(g, []):
                            h(scatters)
                return scatters

            ccs = {}

            # ---- P7 block emitter (defined early; first nblkA blocks are
            # interleaved into conv2 since their gathers only need AG3a+AG3c) ----
            offUP_t = cp.tile([128, 2 * NCH], I32, tag="offUP_t")
            doff = nc.sync.dma_start(out=offUP_t[:], in_=offUP_d[:])
            pacc = cp.tile([128, NCH], F32, tag="pacc")
            nneg = 64 - npos
            p7h = {}
            nblk = (NCH // 8 + 3) // 4  # 31 blocks of up to 4 gchunks

            def emit_p7_block(b, ccdep):
                g0 = b * 4
                gn = min(4, NCH // 8 - g0)
                ncols16 = gn * 16
                tUP = sb.tile([128, 4096], F8, tag="p7_tUP", bufs=2)
                gt_ = nc.gpsimd.indirect_dma_start(
                    out=tUP[:, :ncols16 * 64], out_offset=None, in_=ts_t[:],
                    in_offset=IndirectOffsetOnAxis(
                        ap=offUP_t[:, 16 * g0:16 * g0 + ncols16], axis=0),
                )
                add_dep_helper(gt_.ins, doff.ins, sync=True, reason="gather reads offsets")
                for _c in ccdep:
                    add_dep_helper(gt_.ins, _c.ins, sync=True, reason="gather after AG3")
                if (b - 2) in p7h:
                    add_dep_helper(gt_.ins, p7h[b - 2].ins, sync=True,
                                   reason="WAR tUP slot reuse")
                h8 = sb.tile([128, 2048], BF16, tag="p7_h8", bufs=2)
                a8 = nc.vector.tensor_tensor(
                    out=AP(h8[:].tensor, h8[:].offset,
                           [list(h8[:].ap[0]), [512, gn], [1, 512]]),
                    in0=AP(tUP[:].tensor, tUP[:].offset,
                           [list(tUP[:].ap[0]), [1024, gn], [1, 512]]),
                    in1=AP(tUP[:].tensor, tUP[:].offset + 512,
                           [list(tUP[:].ap[0]), [1024, gn], [1, 512]]),
                    op=ALU.add,
                )
                add_dep_helper(a8.ins, gt_.ins, sync=True, reason="reads tUP")
                p7h[b] = a8
                ncr = gn * 8
                # tables store negated values for the w2<0 dims: the signed
                # reduction is relu on [:npos] + min0 on [npos:], one sum.
                nc.scalar.activation(
                    out=AP(h8[:].tensor, h8[:].offset,
                           [list(h8[:].ap[0]), [64, ncr], [1, npos]]),
                    in_=AP(h8[:].tensor, h8[:].offset,
                           [list(h8[:].ap[0]), [64, ncr], [1, npos]]),
                    func=AF.Relu)
                nc.vector.tensor_scalar_min(
                    out=AP(h8[:].tensor, h8[:].offset + npos,
                           [list(h8[:].ap[0]), [64, ncr], [1, nneg]]),
                    in0=AP(h8[:].tensor, h8[:].offset + npos,
                           [list(h8[:].ap[0]), [64, ncr], [1, nneg]]),
                    scalar1=0.0)
                red = sb.tile([128, 32], F32, tag="p7_red", bufs=3)
                nc.vector.tensor_reduce(
                    out=red[:, :ncr],
                    in_=AP(h8[:].tensor, h8[:].offset,
                           [list(h8[:].ap[0]), [64, ncr], [1, 64]]),
                    axis=mybir.AxisListType.X, op=ALU.add,
                )
                sg = sb.tile([128, 32], F32, tag="p7_sg", bufs=3)
                nc.scalar.activation(out=sg[:, :ncr], in_=red[:, :ncr],
                                     func=AF.Sigmoid, bias=b2pred[:], scale=float(inv_g))
                nc.vector.tensor_scalar_mul(out=pacc[:, g0 * 8:g0 * 8 + ncr],
                                            in0=sg[:, :ncr], scalar1=5.0)

            # band AG hooks: trigger each band's AllGather a few groups after
            # its producers finish. A waiting collective blocks the whole
            # gpsimd queue until it can trigger, so hooks are placed where the
            # CC stream is already free.
            def mk_band_ag(key, src, dst):
                def h(b):
                    def hook(sc):
                        ccs[f"{key}{b}"] = allgather(
                            src, B_OFF[b], B_OFF[b + 1], dst, B_TAB[b],
                            sc[:GEND[b]])
                    return hook
                return h

            def h_cc3c(sc):
                ccs["3c"] = allgather(ag3_in, PU, SHARD, ts_t, PROD_BASE, sc3c)

            ag2h = mk_band_ag("2_", ag2_in, y2_t)
            hooks1 = {2 * i + 1: [lambda sc, g=15 + i: emit_p1c_group(g)]
                      for i in range(10)}
            hooks1[22] = [h_cc3c]
            hooks1[28] = [ag2h(0)]
            hooks1[36] = [ag2h(1)]
            hooks1[44] = [ag2h(2)]
            cv1 = conv_pass(y1_t, lambda c0, w: yown[0:64, c0:c0 + w], 1, ag2_in,
                            cc1, hooks=hooks1)
            ccs["2_3"] = allgather(ag2_in, B_OFF[3], B_OFF[4], y2_t, B_TAB[3], cv1)
            cc2 = [ccs[f"2_{b}"] for b in range(4)]

            # interleave P7 blocks whose ts bands complete mid-conv2 into the
            # conv2 loop (vector/scalar there are mostly idle); the rest run
            # after, band 2 ones ahead of the final band-3 AllGather.
            def p7deps(b, margin=0):
                hi = min(blkband[b] + margin, 3)
                return [ccs["3c"]] + [ccs[f"3_{k}"] for k in range(hi + 1)]

            def mk_p7(b):
                def hook(sc):
                    # inline blocks run right after their band AG completes;
                    # the +1 band margin (~30us) covers remote-write
                    # visibility straggling the AG completion signal.
                    emit_p7_block(b, p7deps(b, margin=1))
                return hook

            band0_blocks = [b for b in range(nblk) if blkband[b] == 0]
            band1_blocks = [b for b in range(nblk) if blkband[b] == 1]
            ag3h = mk_band_ag("3_", ag3_in, ts_t)
            hooks2 = {14: [ag3h(0)], 24: [ag3h(1)], 34: [ag3h(2)]}
            emitted_inline = []
            gslot = 33
            for b in band0_blocks:
                if gslot >= 46:
                    break
                hooks2.setdefault(gslot, []).append(mk_p7(b))
                emitted_inline.append(b)
                gslot += 2
            gslot = max(gslot, 45)
            for b in band1_blocks:
                if gslot >= 49:
                    break
                hooks2.setdefault(gslot, []).append(mk_p7(b))
                emitted_inline.append(b)
                gslot += 2

            cv2 = conv_pass(y2_t, lambda c0, w: yown[64:128, c0:c0 + w], 2, ag3_in,
                            cc2, hooks=hooks2)
            rest = [b for b in range(nblk) if b not in emitted_inline]
            for b in [x for x in rest if blkband[x] <= 1]:
                emit_p7_block(b, p7deps(b))
            ccs["3_3"] = allgather(ag3_in, B_OFF[3], B_OFF[4], ts_t, B_TAB[3], cv2)
            for b in [x for x in rest if blkband[x] >= 2]:
                emit_p7_block(b, p7deps(b))

            nc.sync.dma_start(out=preds_d[:], in_=pacc[:])

    _split_sync_waits(nc)
    return nc


# --------------------------------------------------------------------------
# runner
# --------------------------------------------------------------------------
def _run(inputs, trace=False):
    per_core, shared, meta = _prepare(inputs)
    nc = build_program(meta)
    in_maps = []
    for c in range(N_CORES):
        m = dict(shared)
        m.update(per_core[c])
        in_maps.append({k: np.ascontiguousarray(v) for k, v in m.items()
                        if not k.startswith("_")})
    res = run_bass_kernel_spmd(nc, in_maps, core_ids=list(range(N_CORES)), trace=trace)
    out = np.zeros(NE, np.float32)
    el = np.arange(EPT)
    for c in range(N_CORES):
        pc = res.results[c]["preds"]
        out[c * EPT + per_core[c]["_eorder"][el]] = pc[el % 128, el // 128]
    return out, res.exec_time_ns


def kernel(**inputs):
    out, _ = _run(inputs, trace=False)
    return out



# revision 39
# speedup vs baseline: 1.0225x; 1.0225x over previous
"""Trainium2 Bass kernel for the bipartite GNN recommender (8 NeuronCores).

Redesigned layout (v2):
- Node j -> core j%8. Per-core user rows l=j//8 in [0,25088), products
  l=25088+(p//8) in [25088,37760). Graph edges only touch nodes <200000
  (the reference never offsets prod_idx), so products are self-loop-only
  and their whole chain (proj->conv1->conv2->ts) is computed locally in
  phase P1 with zero collective traffic.
- Conv tables are block-laid per (half, core): user table row for node j:
  l<12544 -> c*12544+l, else 100352+c*12544+(l-12544). One AllGather per
  half, triggered as soon as that half's tiles are produced (overlaps
  compute). Products in ts table at 200704+c*12672+(p//8).
- Scatter segment-sum via transposed one-hot matmuls: stationary = the
  64-col message tile (half the LDWEIGHTS cost), stream = the one-hot,
  accumulate [64,128] per tile in one [64,512] PSUM bank per group.
- Self-term from a persistent SBUF copy of the core's own table slice
  (no indirect gather), dis applied via a persistent [64,25088] bf16
  broadcast table, biases via per-partition activation bias columns.
- Final pair-MLP: |W2| folded into the ts tables (pos/neg dim split on
  host), so per edge: gather t,s -> add -> relu -> two strided reduces
  -> subtract -> sigmoid(scale)*5.
"""
import ml_dtypes
import numpy as np

from concourse import bass, mybir, tile
from concourse.bass import AP, IndirectOffsetOnAxis
from concourse.bass_utils import run_bass_kernel_spmd
from concourse.masks import make_identity
from concourse.tile import add_dep_helper

F32 = mybir.dt.float32
BF16 = mybir.dt.bfloat16
F8 = mybir.dt.float8e4
I32 = mybir.dt.int32

AF = mybir.ActivationFunctionType
ALU = mybir.AluOpType

N_CORES = 8
NU, NP, NE = 200000, 100000, 1000000
SHARD = 37760
PU = 25088            # user rows per core
PC = 12672            # product rows per core
P0 = 12544            # rows per user half per core
UH2 = 8 * P0          # 100352
PROD_BASE = 2 * UH2   # 200704
TAB = PROD_BASE + 8 * PC  # 302080
TILES_C = 99
W32 = 32              # scatter dest-tile width (rows per one-hot tile)
NT = PU // W32        # 784 dest tiles per core
NG = PU // 512        # 49 groups of 16 tiles
EPT = NE // N_CORES
NCH = 984             # pred output cols (125000 edges -> 977, pad to mult of 8)
# user-table bands: AllGathers are split 4 ways and pipelined behind the
# producers; band edges align to 512-row groups (12/12/12/13 groups).
B_OFF = np.array([0, 6144, 12288, 18432, 25088])
B_TAB = 8 * B_OFF     # band base rows in the gathered tables
GEND = [12, 24, 36, 49]  # producing group count per band prefix


# --------------------------------------------------------------------------
# legalization: this walrus build allows at most 1 sync wait per instruction
# --------------------------------------------------------------------------
def _split_sync_waits(nc, max_waits=1):
    import bass_rust
    for bb in nc.main_func.blocks:
        out = []
        for inst in bb.instructions:
            si = inst.sync_info
            if si is not None and si.on_wait is not None and len(si.on_wait) > max_waits:
                waits = list(si.on_wait)
                keep, extra = waits[-max_waits:], waits[:-max_waits]
                while extra:
                    chunk, extra = extra[:max_waits], extra[max_waits:]
                    nop = bass_rust.InstNoOp(name=f"I-{nc.next_id()}", ins=[], outs=[])
                    nop.engine = inst.engine
                    nop.bass_nofuse = True
                    nop.sync_info = mybir.SyncInfo(on_wait=chunk, on_update=[])
                    nc.register_instruction(nop, overwrite=True)
                    out.append(nop)
                si.on_wait = keep
            out.append(inst)
        del bb.instructions[:]
        for i in out:
            bb.add_instruction(i)


# --------------------------------------------------------------------------
# host-side sharding / layout prep
# --------------------------------------------------------------------------
def _pi_user(j):
    j = np.asarray(j, np.int64)
    c, l = j % 8, j // 8
    b = np.searchsorted(B_OFF, l, side="right") - 1
    size = B_OFF[b + 1] - B_OFF[b]
    return (B_TAB[b] + c * size + (l - B_OFF[b])).astype(np.int32)


def _pi_prod(p):
    p = np.asarray(p, np.int64)
    return (PROD_BASE + (p % 8) * PC + p // 8).astype(np.int32)


def _prepare(inputs):
    ei = np.asarray(inputs["edge_index"])
    u_idx = ei[0].astype(np.int64)
    p_idx = ei[1].astype(np.int64)

    # directed messages: src -> dst; both endpoints are node ids < 200000
    src = np.concatenate([u_idx, p_idx])
    dst = np.concatenate([p_idx, u_idx])
    core = (dst % 8).astype(np.int64)
    l = (dst // 8).astype(np.int64)
    src_pi = _pi_user(src)

    order = np.argsort(core * (1 << 32) + l, kind="stable")
    core_s, l_s, srcpi_s = core[order], l[order], src_pi[order]
    core_starts = np.searchsorted(core_s, np.arange(N_CORES + 1))

    # per-tile chunk counts, shared across cores (SPMD: one program)
    cnt = np.bincount(core_s * NT + (l_s // W32),
                      minlength=N_CORES * NT).reshape(N_CORES, NT)
    chunks_t = np.maximum(1, np.ceil(cnt.max(0) / 128).astype(np.int64))
    cb = np.zeros(NT + 1, np.int64)
    np.cumsum(chunks_t, out=cb[1:])
    TC = int(cb[-1])
    # per-group chunk ranges (16 tiles of 32 rows per 512-row group)
    gb = cb[::16]                      # [NG+1] group chunk base
    MAXCH = int(np.max(gb[1:] - gb[:-1]))

    fw = np.asarray(inputs["user_features"], np.float32)
    pw = np.asarray(inputs["product_features"], np.float32)
    ue = np.asarray(inputs["user_emb"], np.float32)
    pe = np.asarray(inputs["product_emb"], np.float32)
    b_uf = np.asarray(inputs["b_uf"], np.float32)
    b_pf = np.asarray(inputs["b_pf"], np.float32)

    pi_u = _pi_user(u_idx)
    pi_p = _pi_prod(p_idx)

    # pred-MLP folding: permute hidden dims so W2>=0 dims come first,
    # scale W1 columns (and pb1) by |W2|*G, recover with sigmoid scale 1/G.
    W1 = np.asarray(inputs["pred_W1"], np.float32)     # [128, 64]
    w2 = np.asarray(inputs["pred_W2"], np.float32).reshape(64)
    pb1 = np.asarray(inputs["pred_b1"], np.float32)
    perm = np.argsort(w2 < 0, kind="stable")           # positives first
    npos = int((w2 >= 0).sum())
    aw = np.abs(w2[perm])
    amax = max(aw.max(), 1e-30)
    G = 1.0 / amax
    colscale = aw * G                                  # in (0, 1]
    W1s = W1[:, perm] * colscale[None, :]
    pb1s = pb1[perm] * colscale
    # negate the w2<0 columns: the stored value v' = -v, so the edge
    # contribution -relu(v) = min(v', 0) and the +/- reduction collapses
    # into ONE contiguous sum (relu on [:npos], min0 on [npos:]).
    W1s[:, npos:] *= -1.0
    pb1s[npos:] *= -1.0
    inv_g = float(amax)                                # sigmoid scale

    per_core = []
    for c in range(N_CORES):
        s0, s1 = core_starts[c], core_starts[c + 1]
        lc, sc = l_s[s0:s1], srcpi_s[s0:s1]
        # flat per-tile chunk layout: tile t owns chunks [cb[t], cb[t+1])
        t = lc // W32
        start = np.searchsorted(t, np.arange(NT))
        pos = np.arange(len(t)) - start[t]
        assert pos.max() < (cb[t + 1] - cb[t]).max() * 128 + 128
        ch = cb[t] + (pos >> 7)
        rows = np.zeros((128, TC), np.int32)
        colv = np.full((128, TC), -1, np.int64)
        rows[pos & 127, ch] = sc
        colv[pos & 127, ch] = lc & (W32 - 1)
        S4 = (colv[:, :, None] == np.arange(W32)
              ).astype(ml_dtypes.float8_e4m3).reshape(128, TC * W32)

        featT = np.zeros((128, SHARD), np.float32)
        embT = np.zeros((64, SHARD), np.float32)
        featT[:, :25000] = fw[c::8].T
        featT[:, 25088:37588] = pw[c::8].T
        embT[:, :25000] = ue[c::8].T + b_uf[:, None]
        embT[:, 25088:37588] = pe[c::8].T + b_pf[:, None]
        embT[:, 25000:25088] = b_uf[:, None]
        embT[:, 37588:] = b_pf[:, None]

        deg = np.bincount(lc, minlength=PU).astype(np.float32)
        dis = 1.0 / np.sqrt(deg + 1.0)
        disTu = np.tile(dis[None, :], (64, 1)).astype(ml_dtypes.bfloat16)

        e0 = c * EPT
        # sort this core's pair-edges by the band of their U-endpoint; a P7
        # block whose edges only touch early bands can start as soon as those
        # band AllGathers complete.
        pu_c = pi_u[e0:e0 + EPT]
        pp_c = pi_p[e0:e0 + EPT]
        uband = np.searchsorted(B_TAB, pu_c, side="right") - 1
        eorder = np.argsort(uband, kind="stable")
        pu_c, pp_c = pu_c[eorder], pp_c[eorder]
        uband_s = uband[eorder]
        blk_band_c = [int(uband_s[min((b + 1) * 4096, EPT) - 1])
                      for b in range((NCH // 8 + 3) // 4)]
        offU = np.zeros((128, NCH), np.int32)
        offP = np.zeros((128, NCH), np.int32)
        el = np.arange(EPT)
        offU[el % 128, el // 128] = pu_c
        offP[el % 128, el // 128] = pp_c
        offUP = np.zeros((128, 2 * NCH), np.int32)
        for g in range(NCH // 8):
            offUP[:, 16 * g:16 * g + 8] = offU[:, 8 * g:8 * g + 8]
            offUP[:, 16 * g + 8:16 * g + 16] = offP[:, 8 * g:8 * g + 8]

        per_core.append(dict(
            featT=featT.astype(ml_dtypes.float8_e4m3), embT=embT.astype(ml_dtypes.bfloat16),
            disTu=disTu, rows=rows, S4=S4,
            offUP=offUP, _colv=colv, _eorder=eorder,
            _blkband=blk_band_c,
        ))

    shared = dict(
        Wuf=np.asarray(inputs["W_uf"], np.float32).astype(ml_dtypes.bfloat16),
        Wpf=np.asarray(inputs["W_pf"], np.float32).astype(ml_dtypes.bfloat16),
        W1c=np.asarray(inputs["conv1_W"], np.float32).astype(ml_dtypes.bfloat16),
        W2c=np.asarray(inputs["conv2_W"], np.float32).astype(ml_dtypes.bfloat16),
        pW1t=np.ascontiguousarray(W1s[:64]).astype(ml_dtypes.bfloat16),
        pW1b=np.ascontiguousarray(W1s[64:]).astype(ml_dtypes.bfloat16),
        b1col=np.asarray(inputs["conv1_b"], np.float32).reshape(64, 1),
        b2col=np.asarray(inputs["conv2_b"], np.float32).reshape(64, 1),
        pb1col=pb1s.reshape(64, 1).astype(np.float32),
        b2pred=np.full((128, 1), float(np.asarray(inputs["pred_b2"]).reshape(())), np.float32),
    )
    blkband = [max(pc["_blkband"][b] for pc in per_core)
               for b in range(len(per_core[0]["_blkband"]))]
    meta = dict(chunks_t=chunks_t.tolist(), cb=cb.tolist(), gb=gb.tolist(),
                TC=TC, MAXCH=MAXCH, npos=npos, inv_g=inv_g, blkband=blkband)
    return per_core, shared, meta


# --------------------------------------------------------------------------
# numpy simulator of the device program (for host-side validation only)
# --------------------------------------------------------------------------
def _simulate(inputs):
    f8 = lambda x: np.asarray(x, np.float32).astype(ml_dtypes.float8_e4m3).astype(np.float32)
    bf = lambda x: np.asarray(x, np.float32).astype(ml_dtypes.bfloat16).astype(np.float32)
    per_core, shared, meta = _prepare(inputs)
    npos, inv_g = meta["npos"], meta["inv_g"]
    cb = np.asarray(meta["cb"])
    Wuf, Wpf = bf(shared["Wuf"]), bf(shared["Wpf"])
    W1c, W2c = bf(shared["W1c"]), bf(shared["W2c"])
    pW1t, pW1b = bf(shared["pW1t"]), bf(shared["pW1b"])
    b1, b2 = shared["b1col"][:, 0], shared["b2col"][:, 0]
    pb1 = shared["pb1col"][:, 0]

    def band_write(tbl, c, arr):
        for b in range(4):
            sz = B_OFF[b + 1] - B_OFF[b]
            tbl[B_TAB[b] + c * sz: B_TAB[b] + (c + 1) * sz] = \
                arr[B_OFF[b]:B_OFF[b + 1]]

    y1_t = np.zeros((PROD_BASE, 64), np.float32)
    ts_t = np.zeros((TAB, 64), np.float32)
    y1ownT, disT, featsT, embsT = [], [], [], []
    for c in range(N_CORES):
        pc = per_core[c]
        ft, et = f8(pc["featT"]), bf(pc["embT"])
        dis = bf(pc["disTu"])[0]  # [PU]
        x0 = bf(ft.T @ Wuf + et.T)              # [SHARD, 64] (user cols valid)
        y1 = f8((x0[:PU] @ W1c) * dis[:, None])
        y1ownT.append(y1)
        disT.append(dis)
        featsT.append(ft)
        embsT.append(et)
        band_write(y1_t, c, y1)
        # region C local chain
        x0c = bf(ft[:, PU:].T @ Wpf + et[:, PU:].T)
        x1c = np.maximum(bf(x0c @ W1c) + b1, 0.0)
        x2c = bf(bf(x1c) @ W2c) + b2
        tsc = f8(bf(x2c) @ pW1b)
        ts_t[PROD_BASE + c * PC: PROD_BASE + (c + 1) * PC] = tsc

    def conv(y_t, layer):
        y2_t = np.zeros((PROD_BASE, 64), np.float32)
        outs = []
        for c in range(N_CORES):
            pc = per_core[c]
            dis = disT[c]
            aggT = np.zeros((64, PU), np.float32)
            rows, colv = pc["rows"], pc["_colv"]
            msg = f8(y_t[rows])                  # [128, TC, 64]
            for t in range(NT):
                acc = np.zeros((64, W32), np.float32)
                for ch in range(cb[t], cb[t + 1]):
                    S = (colv[:, ch:ch + 1] == np.arange(W32)[None, :]
                         ).astype(np.float32)
                    acc += msg[:, ch].T @ S
                aggT[:, t * W32:(t + 1) * W32] = acc
            own = y1ownT[c] if layer == 1 else yown2[c]
            agg = aggT.T + own
            x = bf(agg * dis[:, None])
            if layer == 1:
                x1 = bf(np.maximum(x + b1, 0.0))
                y2 = f8(bf(x1 @ W2c) * dis[:, None])
                outs.append(y2)
                band_write(y2_t, c, y2)
            else:
                x2 = bf(x + b2)
                ts = f8(bf(x2 @ pW1t) + pb1)
                outs.append(ts)
                band_write(ts_t, c, ts)
        return y2_t, outs

    yown2 = None
    y2_t, yown2 = conv(y1_t, 1)
    _, _ = conv(y2_t, 2)

    # P7
    out = np.zeros(NE, np.float32)
    ei = np.asarray(inputs["edge_index"])
    pi_u = _pi_user(ei[0].astype(np.int64))
    pi_p = _pi_prod(ei[1].astype(np.int64))
    t = ts_t[pi_u]
    s = ts_t[pi_p]  # per-edge (device order differs, result order identical)
    v = bf(t + s)
    h = np.concatenate([np.maximum(v[:, :npos], 0.0),
                        np.minimum(v[:, npos:], 0.0)], axis=1)
    logit = h.sum(1)
    z = logit * inv_g + float(np.asarray(inputs["pred_b2"]).reshape(()))
    out[:] = 5.0 / (1.0 + np.exp(-z))
    return out


# --------------------------------------------------------------------------
# device program
# --------------------------------------------------------------------------
def _v3(ap, mid, inner, mid_stride=None, inner_stride=0):
    a = ap.ap
    ms = a[1][0] if mid_stride is None else mid_stride
    return AP(ap.tensor, ap.offset, [list(a[0]), [ms, mid], [inner_stride, inner]])


def _o3(ap, nsub):
    return AP(ap.tensor, ap.offset, [list(ap.ap[0]), [128, nsub], [1, 128]])


def build_program(meta):
    chunks_t = meta["chunks_t"]
    cb = meta["cb"]
    gb = meta["gb"]
    TC, MAXCH = meta["TC"], meta["MAXCH"]
    npos, inv_g = meta["npos"], meta["inv_g"]
    blkband = meta["blkband"]
    nc = bass.Bass("TRN2", target_bir_lowering=False, debug=False, num_devices=N_CORES)

    dp = nc.declare_dram_parameter
    featT_d = dp("featT", [128, SHARD], F8, isOutput=False)
    embT_d = dp("embT", [64, SHARD], BF16, isOutput=False)
    disTu_d = dp("disTu", [64, PU], BF16, isOutput=False)
    rows_d = dp("rows", [128, TC], I32, isOutput=False)
    S4_d = dp("S4", [128, TC * W32], F8, isOutput=False)
    offUP_d = dp("offUP", [128, 2 * NCH], I32, isOutput=False)
    Wuf_d = dp("Wuf", [128, 64], BF16, isOutput=False)
    Wpf_d = dp("Wpf", [128, 64], BF16, isOutput=False)
    W1c_d = dp("W1c", [64, 64], BF16, isOutput=False)
    W2c_d = dp("W2c", [64, 64], BF16, isOutput=False)
    pW1t_d = dp("pW1t", [64, 64], BF16, isOutput=False)
    pW1b_d = dp("pW1b", [64, 64], BF16, isOutput=False)
    b1col_d = dp("b1col", [64, 1], F32, isOutput=False)
    b2col_d = dp("b2col", [64, 1], F32, isOutput=False)
    pb1col_d = dp("pb1col", [64, 1], F32, isOutput=False)
    b2pred_d = dp("b2pred", [128, 1], F32, isOutput=False)
    preds_d = dp("preds", [128, NCH], F32, isOutput=True)

    with tile.TileContext(nc) as tc:
        with tc.tile_pool(name="const", bufs=1) as cp, \
             tc.tile_pool(name="sb", bufs=3) as sb, \
             tc.tile_pool(name="ps", bufs=2, space="PSUM") as ps, \
             tc.tile_pool(name="pssc", bufs=2, space="PSUM") as pssc, \
             tc.tile_pool(name="pst", bufs=2, space="PSUM") as pst:

            def reg_dge(h):
                mloc = nc.lookup_mloc(h)
                if mloc.table_entry_id is None:
                    mloc.table_entry_id = len(nc.dge_table) + 1
                    nc.dge_table.append(mloc.name)
                return h

            ag1_in = reg_dge(nc.dram_tensor("ag1_in", [PU, 64], F8))
            ag2_in = reg_dge(nc.dram_tensor("ag2_in", [PU, 64], F8))
            ag3_in = reg_dge(nc.dram_tensor("ag3_in", [SHARD, 64], F8))
            y1_t = reg_dge(nc.dram_tensor("y1_t", [PROD_BASE, 64], F8, addr_space="Shared"))
            y2_t = reg_dge(nc.dram_tensor("y2_t", [PROD_BASE, 64], F8, addr_space="Shared"))
            ts_t = reg_dge(nc.dram_tensor("ts_t", [TAB, 64], F8, addr_space="Shared"))

            # ---- constants ----
            idn = cp.tile([128, 128], F32, tag="idn")
            make_identity(nc, idn[:])
            idn8 = cp.tile([128, 128], F8, tag="idn8")
            nc.vector.tensor_copy(out=idn8[:], in_=idn[:])
            idn_b = cp.tile([128, 128], BF16, tag="idn_b")
            nc.vector.tensor_copy(out=idn_b[:], in_=idn[:])

            Wuf = cp.tile([128, 64], BF16, tag="Wuf")
            nc.sync.dma_start(out=Wuf[:], in_=Wuf_d[:])
            Wpf = cp.tile([128, 64], BF16, tag="Wpf")
            nc.sync.dma_start(out=Wpf[:], in_=Wpf_d[:])
            W1c = cp.tile([64, 64], BF16, tag="W1c")
            nc.sync.dma_start(out=W1c[:], in_=W1c_d[:])
            W2c = cp.tile([64, 64], BF16, tag="W2c")
            nc.sync.dma_start(out=W2c[:], in_=W2c_d[:])
            pW1t = cp.tile([64, 64], BF16, tag="pW1t")
            nc.sync.dma_start(out=pW1t[:], in_=pW1t_d[:])
            pW1b = cp.tile([64, 64], BF16, tag="pW1b")
            nc.sync.dma_start(out=pW1b[:], in_=pW1b_d[:])
            b1col = cp.tile([64, 1], F32, tag="b1col")
            nc.sync.dma_start(out=b1col[:], in_=b1col_d[:])
            b2col = cp.tile([64, 1], F32, tag="b2col")
            nc.sync.dma_start(out=b2col[:], in_=b2col_d[:])
            pb1col = cp.tile([64, 1], F32, tag="pb1col")
            nc.sync.dma_start(out=pb1col[:], in_=pb1col_d[:])
            b2pred = cp.tile([128, 1], F32, tag="b2pred")
            nc.sync.dma_start(out=b2pred[:], in_=b2pred_d[:])
            # disTu loaded in band pieces interleaved with the first P1 groups
            # so the big transfer does not delay the P1 pipeline start.
            disTu = cp.tile([64, PU], BF16, tag="disTu")
            nc.sync.dma_start(out=disTu[:, :B_OFF[1]], in_=disTu_d[:, :B_OFF[1]])
            rows_all = cp.tile([128, TC], I32, tag="rows_all")
            yown = cp.tile([128, PU], F8, tag="yown")  # rows 0:64 = y1, 64:128 = y2

            def transpose_scatter(srcT, g, dram_out, row0, ident=None):
                """srcT [64, g*128] -> row-major rows [row0, row0+g*128)."""
                trp = pst.tile([128, 256], F32, tag="trp")
                for q in range(g):
                    # transpose via a regular matmul: out = srcT_slice^T @ I
                    nc.tensor.matmul(out=trp[:, q * 64:(q + 1) * 64],
                                     lhsT=srcT[:, q * 128:(q + 1) * 128],
                                     rhs=(ident if ident is not None else idn8)[:64, :64],
                                     start=True, stop=True)
                nnm = sb.tile([128, 256], F8, tag="nnm", bufs=3)
                nc.scalar.activation(out=nnm[:, :g * 64], in_=trp[:, :g * 64], func=AF.Copy)
                d = nc.sync.dma_start(
                    out=AP(dram_out[:].tensor, row0 * 64,
                           [[64, 128], [8192, g], [1, 64]]),
                    in_=AP(nnm[:].tensor, nnm[:].offset,
                           [list(nnm[:].ap[0]), [64, g], [1, 64]]),
                )
                return d

            # ======= P1: user projection + y1 table, P1C interleaved ==========
            sc3c = []

            def emit_p1c_group(g):
                gt = min(4, TILES_C - g * 4)
                col0 = PU + g * 4 * 128
                w = gt * 128
                ft = sb.tile([128, 512], F8, tag="p1c_ft", bufs=3)
                nc.sync.dma_start(out=ft[:, :w], in_=featT_d[:, col0:col0 + w])
                et = sb.tile([64, 512], BF16, tag="p1c_et", bufs=3)
                nc.sync.dma_start(out=et[:, :w], in_=embT_d[:, col0:col0 + w])
                x0p = ps.tile([64, 512], F32, tag="psA")
                nc.tensor.matmul(out=x0p[:, :w], lhsT=Wpf[:], rhs=ft[:, :w], start=True, stop=True)
                x0s = sb.tile([64, 512], BF16, tag="p1c_x0s", bufs=3)
                nc.vector.tensor_add(out=x0s[:, :w], in0=x0p[:, :w], in1=et[:, :w])
                x1p = ps.tile([64, 512], F32, tag="psB", bufs=1)
                nc.tensor.matmul(out=x1p[:, :w], lhsT=W1c[:], rhs=x0s[:, :w], start=True, stop=True)
                x1r = sb.tile([64, 512], BF16, tag="p1c_x1r", bufs=3)
                nc.scalar.activation(out=x1r[:, :w], in_=x1p[:, :w], func=AF.Relu, bias=b1col[:])
                x2p = ps.tile([64, 512], F32, tag="psA")
                nc.tensor.matmul(out=x2p[:, :w], lhsT=W2c[:], rhs=x1r[:, :w], start=True, stop=True)
                x2s = sb.tile([64, 512], BF16, tag="p1c_x2s", bufs=3)
                nc.scalar.activation(out=x2s[:, :w], in_=x2p[:, :w], func=AF.Identity, bias=b2col[:])
                tsp = ps.tile([64, 512], F32, tag="psB", bufs=1)
                nc.tensor.matmul(out=tsp[:, :w], lhsT=pW1b[:], rhs=x2s[:, :w], start=True, stop=True)
                tsc = sb.tile([64, 512], F8, tag="p1c_tsc", bufs=3)
                nc.scalar.activation(out=tsc[:, :w], in_=tsp[:, :w], func=AF.Copy)
                d = transpose_scatter(tsc[:, :w], gt, ag3_in, col0)
                sc3c.append(d)

            sc1 = []  # scatter per group
            for g in range(49):
                col0 = g * 512
                w = 512
                ft = sb.tile([128, 512], F8, tag="p1_ft", bufs=3)
                nc.sync.dma_start(out=ft[:], in_=featT_d[:, col0:col0 + w])
                et = sb.tile([64, 512], BF16, tag="p1_et", bufs=3)
                nc.sync.dma_start(out=et[:], in_=embT_d[:, col0:col0 + w])
                if 1 <= g <= 3:  # stream the rest of disTu behind the pipeline
                    nc.sync.dma_start(out=disTu[:, B_OFF[g]:B_OFF[g + 1]],
                                      in_=disTu_d[:, B_OFF[g]:B_OFF[g + 1]])
                x0p = ps.tile([64, 512], F32, tag="psA")
                nc.tensor.matmul(out=x0p[:], lhsT=Wuf[:], rhs=ft[:], start=True, stop=True)
                x0s = sb.tile([64, 512], BF16, tag="p1_x0s", bufs=3)
                nc.vector.tensor_add(out=x0s[:], in0=x0p[:], in1=et[:])
                y1p = ps.tile([64, 512], F32, tag="psB", bufs=1)
                nc.tensor.matmul(out=y1p[:], lhsT=W1c[:], rhs=x0s[:], start=True, stop=True)
                y1s = sb.tile([64, 512], BF16, tag="p1_y1s", bufs=3)
                nc.vector.tensor_tensor(out=y1s[:], in0=y1p[:],
                                        in1=disTu[:, col0:col0 + w], op=ALU.mult)
                nc.scalar.activation(out=yown[0:64, col0:col0 + w], in_=y1s[:],
                                     func=AF.Copy)
                d = transpose_scatter(y1s[:], 4, ag1_in, col0, ident=idn_b)
                sc1.append(d)

            # conv scatter offsets + P7 offsets: loaded during the AG1 window
            drows = nc.sync.dma_start(out=rows_all[:], in_=rows_d[:])

            def allgather(src, r0, r1, dst, o0, scatters):
                cc = nc.gpsimd.collective_compute(
                    "AllGather", ALU.bypass,
                    ins=[src[r0:r1, :]],
                    outs=[dst[o0:o0 + N_CORES * (r1 - r0), :]],
                    replica_groups=[list(range(N_CORES))],
                )
                for s in scatters:
                    add_dep_helper(cc.ins, s.ins, sync=True, reason="AG reads scatters")
                return cc

            # band AllGathers fire as soon as their producer groups finish
            cc1 = [allgather(ag1_in, B_OFF[b], B_OFF[b + 1], y1_t, B_TAB[b],
                             sc1[:GEND[b]]) for b in range(4)]
            # product chain fills the AG1 wait window
            for g in range(25):
                emit_p1c_group(g)

            # ================= conv passes =================
            # per-tile chunk counts (32-row dest tiles, 16 tiles per group)
            def conv_pass(yt, yprevT, layer, ag_out, cc_dep, hooks=None):
                scatters = []
                hist = {}
                for g in range(NG):
                    c0 = gb[g]
                    ncols = gb[g + 1] - c0
                    msg = sb.tile([128, MAXCH * 64], F8, tag="cv_msg", bufs=6)
                    gm = nc.gpsimd.indirect_dma_start(
                        out=msg[:, :ncols * 64], out_offset=None,
                        in_=yt[:],
                        in_offset=IndirectOffsetOnAxis(
                            ap=rows_all[:, c0:c0 + ncols], axis=0),
                    )
                    add_dep_helper(gm.ins, drows.ins, sync=True,
                                   reason="gather reads offsets")
                    for _c in cc_dep:
                        add_dep_helper(gm.ins, _c.ins, sync=True,
                                       reason="gather after AG")
                    if (g - 6) in hist:
                        add_dep_helper(gm.ins, hist[g - 6].ins, sync=True,
                                       reason="WAR msg slot reuse")
                    # host-built one-hot matrices, streamed from DRAM
                    s4a = sb.tile([128, MAXCH * W32], F8, tag="cv_s4", bufs=5)
                    nc.sync.dma_start(out=s4a[:, :ncols * W32],
                                      in_=S4_d[:, c0 * W32:(c0 + ncols) * W32])
                    scp = pssc.tile([64, 512], F32, tag="cv_scp", bufs=3)
                    mm = None
                    mmi = 0
                    msgt, msgo = msg[:].tensor, msg[:].offset
                    s4t, s4o = s4a[:].tensor, s4a[:].offset
                    mpart = list(msg[:].ap[0])
                    spart = list(s4a[:].ap[0])
                    for t in range(16):
                        nchv = chunks_t[16 * g + t]
                        tb = cb[16 * g + t] - c0
                        for j2 in range(nchv // 2):
                            ch = tb + 2 * j2
                            mm = nc.tensor.matmul(
                                out=scp[:, t * W32:(t + 1) * W32],
                                lhsT=AP(msgt, msgo + ch * 64,
                                        [mpart, [64, 2], [1, 64]]),
                                rhs=AP(s4t, s4o + ch * W32,
                                       [spart, [W32, 2], [1, W32]]),
                                start=(j2 == 0),
                                stop=(j2 == nchv // 2 - 1 and nchv % 2 == 0),
                                perf_mode=mybir.MatmulPerfMode.DoubleRow,
                            )
                            if mmi < 2:
                                # the wait gates LDWEIGHTS prefetch too; once
                                # two in-order matmuls carry it, the rest of
                                # the group is implicitly ordered after the
                                # gather (engine queues are in-order).
                                add_dep_helper(mm.ins, gm.ins, sync=True,
                                               reason="matmul reads gathered msg")
                            mmi += 1
                        if nchv % 2:
                            ch = tb + nchv - 1
                            mm = nc.tensor.matmul(
                                out=scp[:, t * W32:(t + 1) * W32],
                                lhsT=AP(msgt, msgo + ch * 64, [mpart, [1, 64]]),
                                rhs=AP(s4t, s4o + ch * W32, [spart, [1, W32]]),
                                start=(nchv == 1), stop=True,
                            )
                            if mmi < 2:
                                add_dep_helper(mm.ins, gm.ins, sync=True,
                                               reason="matmul reads gathered msg")
                            mmi += 1
                    hist[g] = mm
                    col0 = g * 512
                    w = 512
                    t1 = sb.tile([64, 512], F32, tag="cv_t1", bufs=3)
                    nc.vector.tensor_add(out=t1[:, :w], in0=scp[:, :w],
                                         in1=yprevT(col0, w))
                    t2 = sb.tile([64, 512], BF16, tag="cv_t2", bufs=3)
                    nc.vector.tensor_tensor(out=t2[:, :w], in0=t1[:, :w],
                                            in1=disTu[:, col0:col0 + w], op=ALU.mult)
                    if layer == 1:
                        x1r = sb.tile([64, 512], BF16, tag="cv_x1r", bufs=3)
                        nc.scalar.activation(out=x1r[:, :w], in_=t2[:, :w],
                                             func=AF.Relu, bias=b1col[:])
                        y2p = ps.tile([64, 512], F32, tag="psB", bufs=1)
                        nc.tensor.matmul(out=y2p[:, :w], lhsT=W2c[:], rhs=x1r[:, :w],
                                         start=True, stop=True)
                        y2s = sb.tile([64, 512], BF16, tag="cv_y2s", bufs=3)
                        nc.vector.tensor_tensor(out=y2s[:, :w], in0=y2p[:, :w],
                                                in1=disTu[:, col0:col0 + w], op=ALU.mult)
                        nc.scalar.activation(out=yown[64:128, col0:col0 + w],
                                             in_=y2s[:, :w], func=AF.Copy)
                        d = transpose_scatter(y2s[:, :w], 4, ag_out, col0, ident=idn_b)
                    else:
                        x2s = sb.tile([64, 512], BF16, tag="cv_x2s", bufs=3)
                        nc.scalar.activation(out=x2s[:, :w], in_=t2[:, :w],
                                             func=AF.Identity, bias=b2col[:])
                        tsp = ps.tile([64, 512], F32, tag="psB", bufs=1)
                        nc.tensor.matmul(out=tsp[:, :w], lhsT=pW1t[:], rhs=x2s[:, :w],
                                         start=True, stop=True)
                        tsu = sb.tile([64, 512], F8, tag="cv_tsu", bufs=3)
                        nc.scalar.activation(out=tsu[:, :w], in_=tsp[:, :w],
                                             func=AF.Identity, bias=pb1col[:])
                        d = transpose_scatter(tsu[:, :w], 4, ag_out, col0)
                    scatters.append(d)
                    if hooks and g in hooks:
                        hooks[g](scatters)
                return scatters

            ccs = {}

            # ---- P7 block emitter (defined early; first nblkA blocks are
            # interleaved into conv2 since their gathers only need AG3a+AG3c) ----
            offUP_t = cp.tile([128, 2 * NCH], I32, tag="offUP_t")
            doff = nc.sync.dma_start(out=offUP_t[:], in_=offUP_d[:])
            pacc = cp.tile([128, NCH], F32, tag="pacc")
            nneg = 64 - npos
            p7h = {}
            nblk = (NCH // 8 + 3) // 4  # 31 blocks of up to 4 gchunks

            def emit_p7_block(b, ccdep):
                g0 = b * 4
                gn = min(4, NCH // 8 - g0)
                ncols16 = gn * 16
                tUP = sb.tile([128, 4096], F8, tag="p7_tUP", bufs=2)
                gt_ = nc.gpsimd.indirect_dma_start(
                    out=tUP[:, :ncols16 * 64], out_offset=None, in_=ts_t[:],
                    in_offset=IndirectOffsetOnAxis(
                        ap=offUP_t[:, 16 * g0:16 * g0 + ncols16], axis=0),
                )
                add_dep_helper(gt_.ins, doff.ins, sync=True, reason="gather reads offsets")
                for _c in ccdep:
                    add_dep_helper(gt_.ins, _c.ins, sync=True, reason="gather after AG3")
                if (b - 2) in p7h:
                    add_dep_helper(gt_.ins, p7h[b - 2].ins, sync=True,
                                   reason="WAR tUP slot reuse")
                h8 = sb.tile([128, 2048], BF16, tag="p7_h8", bufs=2)
                a8 = nc.vector.tensor_tensor(
                    out=AP(h8[:].tensor, h8[:].offset,
                           [list(h8[:].ap[0]), [512, gn], [1, 512]]),
                    in0=AP(tUP[:].tensor, tUP[:].offset,
                           [list(tUP[:].ap[0]), [1024, gn], [1, 512]]),
                    in1=AP(tUP[:].tensor, tUP[:].offset + 512,
                           [list(tUP[:].ap[0]), [1024, gn], [1, 512]]),
                    op=ALU.add,
                )
                add_dep_helper(a8.ins, gt_.ins, sync=True, reason="reads tUP")
                p7h[b] = a8
                ncr = gn * 8
                # tables store negated values for the w2<0 dims: the signed
                # reduction is relu on [:npos] + min0 on [npos:], one sum.
                nc.scalar.activation(
                    out=AP(h8[:].tensor, h8[:].offset,
                           [list(h8[:].ap[0]), [64, ncr], [1, npos]]),
                    in_=AP(h8[:].tensor, h8[:].offset,
                           [list(h8[:].ap[0]), [64, ncr], [1, npos]]),
                    func=AF.Relu)
                nc.vector.tensor_scalar_min(
                    out=AP(h8[:].tensor, h8[:].offset + npos,
                           [list(h8[:].ap[0]), [64, ncr], [1, nneg]]),
                    in0=AP(h8[:].tensor, h8[:].offset + npos,
                           [list(h8[:].ap[0]), [64, ncr], [1, nneg]]),
                    scalar1=0.0)
                red = sb.tile([128, 32], F32, tag="p7_red", bufs=3)
                nc.vector.tensor_reduce(
                    out=red[:, :ncr],
                    in_=AP(h8[:].tensor, h8[:].offset,
                           [list(h8[:].ap[0]), [64, ncr], [1, 64]]),
                    axis=mybir.AxisListType.X, op=ALU.add,
                )
                sg = sb.tile([128, 32], F32, tag="p7_sg", bufs=3)
                nc.scalar.activation(out=sg[:, :ncr], in_=red[:, :ncr],
                                     func=AF.Sigmoid, bias=b2pred[:], scale=float(inv_g))
                nc.vector.tensor_scalar_mul(out=pacc[:, g0 * 8:g0 * 8 + ncr],
                                            in0=sg[:, :ncr], scalar1=5.0)

            # band AG hooks: trigger each band's AllGather a few groups after
            # its producers finish. A waiting collective blocks the whole
            # gpsimd queue until it can trigger, so hooks are placed where the
            # CC stream is already free.
            def mk_band_ag(key, src, dst):
                def h(b):
                    def hook(sc):
                        ccs[f"{key}{b}"] = allgather(
                            src, B_OFF[b], B_OFF[b + 1], dst, B_TAB[b],
                            sc[:GEND[b]])
                    return hook
                return h

            def h_cc3c(sc):
                ccs["3c"] = allgather(ag3_in, PU, SHARD, ts_t, PROD_BASE, sc3c)

            ag2h = mk_band_ag("2_", ag2_in, y2_t)
            cv1 = conv_pass(y1_t, lambda c0, w: yown[0:64, c0:c0 + w], 1, ag2_in,
                            cc1, hooks={8: h_cc3c, 20: ag2h(0), 30: ag2h(1),
                                        40: ag2h(2)})
            ccs["2_3"] = allgather(ag2_in, B_OFF[3], B_OFF[4], y2_t, B_TAB[3], cv1)
            cc2 = [ccs[f"2_{b}"] for b in range(4)]

            # interleave P7 blocks whose ts bands complete mid-conv2 into the
            # conv2 loop (vector/scalar there are mostly idle); the rest run
            # after, band 2 ones ahead of the final band-3 AllGather.
            def p7deps(b, margin=0):
                hi = min(blkband[b] + margin, 3)
                return [ccs["3c"]] + [ccs[f"3_{k}"] for k in range(hi + 1)]

            def mk_p7(b):
                def hook(sc):
                    # inline blocks run right after their band AG completes;
                    # the +1 band margin (~30us) covers remote-write
                    # visibility straggling the AG completion signal.
                    emit_p7_block(b, p7deps(b, margin=1))
                return hook

            band0_blocks = [b for b in range(nblk) if blkband[b] == 0]
            ag3h = mk_band_ag("3_", ag3_in, ts_t)
            hooks2 = {20: ag3h(0), 30: ag3h(1), 40: ag3h(2)}
            emitted_inline = []
            gslot = 37
            for b in band0_blocks:
                if gslot >= 48:
                    break
                hooks2[gslot] = mk_p7(b)
                emitted_inline.append(b)
                gslot += 2

            cv2 = conv_pass(y2_t, lambda c0, w: yown[64:128, c0:c0 + w], 2, ag3_in,
                            cc2, hooks=hooks2)
            rest = [b for b in range(nblk) if b not in emitted_inline]
            for b in [x for x in rest if blkband[x] <= 1]:
                emit_p7_block(b, p7deps(b))
            ccs["3_3"] = allgather(ag3_in, B_OFF[3], B_OFF[4], ts_t, B_TAB[3], cv2)
            for b in [x for x in rest if blkband[x] >= 2]:
                emit_p7_block(b, p7deps(b))

            nc.sync.dma_start(out=preds_d[:], in_=pacc[:])

    _split_sync_waits(nc)
    return nc


# --------------------------------------------------------------------------
# runner
# --------------------------------------------------------------------------
def _run(inputs, trace=False):
    per_core, shared, meta = _prepare(inputs)
    nc = build_program(meta)
    in_maps = []
    for c in range(N_CORES):
        m = dict(shared)
        m.update(per_core[c])
        in_maps.append({k: np.ascontiguousarray(v) for k, v in m.items()
                        if not k.startswith("_")})
    res = run_bass_kernel_spmd(nc, in_maps, core_ids=list(range(N_CORES)), trace=trace)
    out = np.zeros(NE, np.float32)
    el = np.arange(EPT)
    for c in range(N_CORES):
        pc = res.results[c]["preds"]
        out[c * EPT + per_core[c]["_eorder"][el]] = pc[el % 128, el // 128]
    return out, res.exec_time_ns


def kernel(**inputs):
    out, _ = _run(inputs, trace=False)
    return out

